# revision 12
# baseline (speedup 1.0000x reference)
"""Trainium2 Bass kernel: 4-layer MLP (784-512-512-512-10) + log_softmax.

Data-parallel over 8 NeuronCores: batch 65536 is split into 8 shards of
8192 rows; the ~1M-param weights are replicated on every core.

v4 schedule: batch-half-major pipeline with 1-bank PSUM groups.
  * Every (m-chunk, batch-half) matmul group accumulates into its own
    2KB PSUM bank (pool of 8); consumers run one half-block (~3us)
    behind producers, so neither the PE nor its LDWEIGHTS ever waits on
    a ReLU drain (the v3 failure mode: 4 simultaneously-opened 2-bank
    groups stalled the in-order PE queue on PSUM WAR ~1.5us/superchunk
    and the micro-gaps re-throttled the clock to 1.2 GHz).
  * Layer 1's K=16 remainder opens each group via row-tiled
    (tile_position=(32m,0)) matmuls, 4 running concurrently in distinct
    PE row-groups, instead of burning 8 full 512-cycle slots.
  * bias+ReLU alternates ScalarE/DVE per (m, half) so both engines stay
    under the PE's pace; exp/sum-of-exp for superchunk sc-1 is issued
    mid-superchunk (never ahead of ReLUs the PE needs); ln+subtract+
    store happen in two bulk epilogues (ScalarE activation-table swaps
    for LN cost 1.3us each, so they must not recur per superchunk).
  * Layer 4 runs one superchunk behind; all DRAM buffers host-packed
    for contiguous-per-partition DMA; output leaves in the flat SBUF
    layout [128, 64, 10] and is un-permuted on host.

Measured on axon trn2: 172.8us (v1 baseline) -> 155.2us (v3) -> v4.
"""

from contextlib import ExitStack

import ml_dtypes
import numpy as np

import concourse.bass as bass  # noqa: F401  (registers AP machinery)
from concourse import bacc, mybir
from concourse.bass_utils import run_bass_kernel_spmd
from concourse.tile import TileContext

BF16 = mybir.dt.bfloat16
FP32 = mybir.dt.float32
FP8 = mybir.dt.float8e4

N_CORES = 8
B = 65536
D0, H, C = 784, 512, 10
BC = B // N_CORES            # 8192 rows per core
NB = 512                     # matmul moving free dim / PSUM bank width
HB = 2                       # batch halves per superchunk
SNB = NB * HB                # 1024-row superchunk
NCHUNK = BC // SNB           # 8 superchunks
K0F = 6                      # full 128-row contraction chunks in layer 1
K0R = D0 - K0F * 128         # 16 remainder rows
KH = H // 128                # 4 contraction chunks for hidden layers
MG = SNB // 128              # 8 row-groups per superchunk
NRG = BC // 128              # 64 row-groups of 128 rows per core

_CACHED_NC = None


def build_nc():
    nc = bacc.Bacc(
        "TRN2",
        target_bir_lowering=False,
        debug=False,
        enable_asserts=False,
        num_devices=N_CORES,
    )
    xm_d = nc.declare_dram_parameter("xmain", [NCHUNK * 128, K0F * SNB], FP8, isOutput=False)
    xr_d = nc.declare_dram_parameter("xrem", [NCHUNK * 128, SNB], FP8, isOutput=False)
    w1_d = nc.declare_dram_parameter("w1p", [128, K0F * H], FP8, isOutput=False)
    w1r_d = nc.declare_dram_parameter("w1r", [128, 128], FP8, isOutput=False)
    w2_d = nc.declare_dram_parameter("w2p", [128, KH * H], FP8, isOutput=False)
    w3_d = nc.declare_dram_parameter("w3p", [128, KH * H], FP8, isOutput=False)
    w4_d = nc.declare_dram_parameter("w4p", [128, KH * C], BF16, isOutput=False)
    bal_d = nc.declare_dram_parameter("ball", [128, 3 * KH + C], FP32, isOutput=False)
    out_d = nc.declare_dram_parameter("out", [128, NRG, C], FP32, isOutput=True)

    expf = mybir.ActivationFunctionType.Exp
    reluf = mybir.ActivationFunctionType.Relu
    lnf = mybir.ActivationFunctionType.Ln
    add_op = mybir.AluOpType.add
    max_op = mybir.AluOpType.max
    sub_op = mybir.AluOpType.subtract
    drow = mybir.MatmulPerfMode.DoubleRow

    with TileContext(nc) as tc, ExitStack() as ctx:
        consts = ctx.enter_context(tc.tile_pool(name="consts", bufs=1))
        xpool = ctx.enter_context(tc.tile_pool(name="xp", bufs=3))
        hpool = ctx.enter_context(tc.tile_pool(name="hp", bufs=3))
        spool = ctx.enter_context(tc.tile_pool(name="sp", bufs=2))
        pbig = ctx.enter_context(tc.tile_pool(name="pbig", bufs=8, space="PSUM"))

        # Weight-queue order = first-use order: the row-tiled remainder
        # weights open superchunk 0, then w1, then biases (first ReLU is
        # ~2us later), then the rest.
        w1r = consts.tile([128, 128], FP8, tag="w1r", name="w1r")
        nc.scalar.dma_start(w1r[:], w1r_d[:])
        w1 = consts.tile([128, K0F, H], FP8, tag="w1", name="w1")
        for j in range(K0F // 2):
            # k-pair-sized chunks so superchunk 0's kp-major matmuls can
            # start as soon as the first pair lands.
            nc.scalar.dma_start(
                w1[:, 2 * j : 2 * j + 2, :], w1_d[:, j * 2 * H : (j + 1) * 2 * H]
            )
        ball = consts.tile([128, 3 * KH + C], FP32, tag="ball", name="ball")
        nc.scalar.dma_start(ball[:], bal_d[:])
        w2 = consts.tile([128, KH, H], FP8, tag="w2", name="w2")
        nc.scalar.dma_start(w2[:], w2_d[:])
        w3 = consts.tile([128, KH, H], FP8, tag="w3", name="w3")
        nc.scalar.dma_start(w3[:], w3_d[:])
        w4 = consts.tile([128, KH, C], BF16, tag="w4", name="w4")
        nc.scalar.dma_start(w4[:], w4_d[:])

        b4s = ball[:, 3 * KH : 3 * KH + C]

        # PE warm-up: dummy matmuls during the initial DMA wait so the HAM
        # clock gate is at 2.4 GHz when real work arrives.
        warm = consts.tile([128, NB], FP8, tag="warm", name="warm")
        nc.vector.memset(warm[:], 1.0)
        psw = pbig.tile([128, NB], FP32, tag="ps", name="ps_warm")
        for i in range(6):
            nc.tensor.matmul(
                psw[:], lhsT=warm[:, 0:128], rhs=warm[:],
                start=(i == 0), stop=(i == 5),
            )

        # Persistent softmax state: ln+subtract are deferred to two bulk
        # epilogue calls, so ScalarE never swaps activation tables (RELU/EXP
        # share a set, LN does not) inside the steady-state loop.
        logits_all = consts.tile([128, NRG, C], FP32, tag="logits_all", name="logits_all")
        esum_all = consts.tile([128, NRG], FP32, tag="esum_all", name="esum_all")
        lns_all = consts.tile([128, NRG], FP32, tag="lns_all", name="lns_all")
        obuf = consts.tile([128, NRG, C], FP32, tag="obuf", name="obuf")

        def relu_half(ps, out, bias_ap, on_scalar):
            if on_scalar:
                nc.scalar.activation(out, ps[:], reluf, bias=bias_ap)
            else:
                nc.vector.tensor_scalar(out, ps[:], bias_ap, 0.0, add_op, max_op)

        def l4_matmuls(h3, ps4):
            for hb in range(HB):
                for mm in range(NB // 128):
                    r = hb * (NB // 128) + mm
                    ms = slice(mm * 128, (mm + 1) * 128)
                    for k in range(KH):
                        nc.tensor.matmul(
                            ps4[:, r, :], lhsT=h3[k][:, hb, ms], rhs=w4[:, k, :],
                            start=(k == 0), stop=(k == KH - 1),
                        )

        def l4_softmax_state(sc, ps4):
            # logits + exp + sum(exp) for superchunk sc.
            rg0 = sc * MG
            lg = logits_all[:, rg0 : rg0 + MG, :]
            nc.vector.tensor_tensor(
                lg, ps4[:, 0:MG, :],
                b4s[:, None, :].to_broadcast((128, MG, C)), add_op,
            )
            etile = spool.tile([128, MG, C], FP32, tag="etile", name="etile")
            nc.scalar.activation(etile[:], lg, expf)
            nc.vector.tensor_reduce(
                esum_all[:, rg0 : rg0 + MG], etile[:],
                axis=mybir.AxisListType.X, op=add_op,
            )

        def softmax_epilogue(rg0, rg1):
            # out = logits - ln(sum(exp(logits))) for row-groups [rg0, rg1)
            n = rg1 - rg0
            nc.scalar.activation(lns_all[:, rg0:rg1], esum_all[:, rg0:rg1], lnf)
            nc.vector.tensor_tensor(
                obuf[:, rg0:rg1, :], logits_all[:, rg0:rg1, :],
                lns_all[:, rg0:rg1, None].to_broadcast((128, n, C)), sub_op,
            )
            nc.sync.dma_start(out_d[:, rg0:rg1, :], obuf[:, rg0:rg1, :])

        h3_prev = None
        ps4_prev = None

        for sc in range(NCHUNK):
            xr = xpool.tile([128, SNB], FP8, tag="xr", name="xr")
            nc.sync.dma_start(xr[:], xr_d[sc * 128 : (sc + 1) * 128, :])
            xt = xpool.tile([128, K0F, SNB], FP8, tag="xt", name="xt")
            for j in range(K0F // 2):
                nc.sync.dma_start(
                    xt[:, 2 * j : 2 * j + 2, :],
                    xm_d[sc * 128 : (sc + 1) * 128, j * 2 * SNB : (j + 1) * 2 * SNB],
                )

            # Layer 1 [784 -> 512], one batch-half at a time: the K=16
            # remainder opens all 4 m-groups concurrently (distinct PE
            # row-groups), then 3 fp8-DoubleRow slots per m.
            h1p = [
                hpool.tile([128, 2, HB, NB], FP8, tag=f"h1p_{j}", name=f"h1p_{j}")
                for j in range(KH // 2)
            ]
            for hb in range(HB):
                bsl = slice(hb * NB, (hb + 1) * NB)
                ps1 = [
                    pbig.tile([128, NB], FP32, tag="ps", name=f"ps1_{m}_{hb}")
                    for m in range(KH)
                ]
                for m in range(KH):
                    nc.tensor.matmul(
                        ps1[m][:], lhsT=w1r[32 * m : 32 * m + K0R, :],
                        rhs=xr[32 * m : 32 * m + K0R, bsl],
                        start=True, stop=False, perf_mode=None,
                        tile_position=(32 * m, 0),
                    )
                # kp-major so superchunk 0 can start on the first-arriving
                # x/w1 k-pair chunk.
                for k in range(0, K0F, 2):
                    for m in range(KH):
                        ms = slice(m * 128, (m + 1) * 128)
                        nc.tensor.matmul(
                            ps1[m][:], lhsT=w1[:, k : k + 2, ms],
                            rhs=xt[:, k : k + 2, bsl],
                            start=False, stop=(k == K0F - 2), perf_mode=drow,
                        )
                for m in range(KH):
                    relu_half(
                        ps1[m], h1p[m // 2][:, m % 2, hb, :],
                        ball[:, m : m + 1], on_scalar=((m + hb) % 2 == 0),
                    )

            # Layer 4 of the previous superchunk (its inputs are long ready).
            if h3_prev is not None:
                l4_matmuls(h3_prev, ps4_prev)

            def hidden_layer(w, src, dsts, bias_base, out_of_h3):
                for hb in range(HB):
                    ps = [
                        pbig.tile([128, NB], FP32, tag="ps", name=f"psh_{m}_{hb}")
                        for m in range(KH)
                    ]
                    for m in range(KH):
                        ms = slice(m * 128, (m + 1) * 128)
                        for j in range(KH // 2):
                            nc.tensor.matmul(
                                ps[m][:], lhsT=w[:, 2 * j : 2 * j + 2, ms],
                                rhs=src[j][:, :, hb, :],
                                start=(j == 0), stop=(j == KH // 2 - 1),
                                perf_mode=drow,
                            )
                        out = (
                            dsts[m][:, hb, :] if out_of_h3
                            else dsts[m // 2][:, m % 2, hb, :]
                        )
                        relu_half(
                            ps[m], out, ball[:, bias_base + m : bias_base + m + 1],
                            on_scalar=((m + hb) % 2 == 0),
                        )
                    if hb == 0 and out_of_h3 is False and h3_prev is not None:
                        # exp/sum(exp) of the previous superchunk: issued
                        # mid-superchunk so its ScalarE/DVE ops never queue
                        # ahead of ReLUs the PE is about to wait on.
                        l4_softmax_state(sc - 1, ps4_prev)
                        if sc == NCHUNK - 1:
                            # Bulk ln+subtract+store for superchunks 0-6;
                            # the L3 consumers of the ReLUs this delays are
                            # a full half-block behind, so the activation-
                            # table swap hides here.
                            softmax_epilogue(0, (NCHUNK - 1) * MG)

            # Layer 2 [512 -> 512]
            h2p = [
                hpool.tile([128, 2, HB, NB], FP8, tag=f"h2p_{j}", name=f"h2p_{j}")
                for j in range(KH // 2)
            ]
            hidden_layer(w2, h1p, h2p, KH, out_of_h3=False)

            # Layer 3 [512 -> 512], bf16 out (layer-4 lhsT)
            h3 = [
                hpool.tile([128, HB, NB], BF16, tag=f"h3_{m}", name=f"h3_{m}")
                for m in range(KH)
            ]
            hidden_layer(w3, h2p, h3, 2 * KH, out_of_h3=True)

            h3_prev = h3
            ps4_prev = pbig.tile([128, MG, C], FP32, tag="ps", name="ps4")

        l4_matmuls(h3_prev, ps4_prev)
        l4_softmax_state(NCHUNK - 1, ps4_prev)
        softmax_epilogue((NCHUNK - 1) * MG, NRG)

    nc.compile()
    return nc


def _get_nc():
    global _CACHED_NC
    if _CACHED_NC is None:
        _CACHED_NC = build_nc()
    return _CACHED_NC


def make_in_maps(x, W1, b1, W2, b2, W3, b3, W4, b4):
    bf16 = ml_dtypes.bfloat16
    fp8 = ml_dtypes.float8_e4m3
    f32 = np.float32
    W1, W2, W3, W4 = (np.asarray(w, dtype=f32) for w in (W1, W2, W3, W4))

    # w1p[p, k*512+m] = W1[m, k*128+p]
    w1p = np.ascontiguousarray(
        W1[:, : K0F * 128].reshape(H, K0F, 128).transpose(2, 1, 0)
    ).reshape(128, K0F * H).astype(fp8)
    # w1r[32i+j, c] = W1[128i+c, 768+j]  (row-tiled remainder weights)
    w1r = np.zeros((128, 128), dtype=fp8)
    wr = W1[:, K0F * 128 :].astype(fp8)  # [512, 16]
    for i in range(KH):
        w1r[32 * i : 32 * i + K0R, :] = wr[128 * i : 128 * (i + 1), :].T
    # w2p[p, o*512+m] = W2[m, o*128+p]
    def packw(W):
        return np.ascontiguousarray(
            W.T.reshape(KH, 128, H).transpose(1, 0, 2)
        ).reshape(128, KH * H).astype(fp8)
    w2p, w3p = packw(W2), packw(W3)
    w4p = np.ascontiguousarray(
        W4.T.reshape(KH, 128, C).transpose(1, 0, 2)
    ).reshape(128, KH * C).astype(bf16)
    ball = np.concatenate(
        [
            np.asarray(b1, f32).reshape(KH, 128).T,
            np.asarray(b2, f32).reshape(KH, 128).T,
            np.asarray(b3, f32).reshape(KH, 128).T,
            np.tile(np.asarray(b4, f32)[None, :], (128, 1)),
        ],
        axis=1,
    )
    common = {
        "w1p": w1p, "w1r": w1r, "w2p": w2p, "w3p": w3p, "w4p": w4p,
        "ball": np.ascontiguousarray(ball),
    }

    xq = np.asarray(x).astype(fp8)
    in_maps = []
    for ci in range(N_CORES):
        xs = xq[ci * BC : (ci + 1) * BC]  # [8192, 784]
        # xmain[sc*128+p, k*1024+b] = xs[sc*1024+b, k*128+p]
        xmain = np.ascontiguousarray(
            xs[:, : K0F * 128].reshape(NCHUNK, SNB, K0F, 128).transpose(0, 3, 2, 1)
        ).reshape(NCHUNK * 128, K0F * SNB)
        # xrem[sc*128+32i+j, b] = xs[sc*1024+b, 768+j], replicated over i
        xrp = xs[:, K0F * 128 :].reshape(NCHUNK, SNB, K0R).transpose(0, 2, 1)
        xrem = np.zeros((NCHUNK, 128, SNB), dtype=fp8)
        for i in range(KH):
            xrem[:, 32 * i : 32 * i + K0R, :] = xrp
        in_maps.append(
            {"xmain": xmain, "xrem": xrem.reshape(NCHUNK * 128, SNB), **common}
        )
    return in_maps


def assemble_output(res):
    # out dram is the flat SBUF layout [128, 64, 10]; row rg*128+p of the
    # core's shard lives at out[p, rg, :].
    parts = []
    for i in range(N_CORES):
        o = np.asarray(res.results[i]["out"], dtype=np.float32)
        parts.append(o.transpose(1, 0, 2).reshape(BC, C))
    return np.concatenate(parts, axis=0)


def kernel(x, W1, b1, W2, b2, W3, b3, W4, b4):
    in_maps = make_in_maps(x, W1, b1, W2, b2, W3, b3, W4, b4)
    nc = _get_nc()
    res = run_bass_kernel_spmd(nc, in_maps, list(range(N_CORES)))
    return assemble_output(res)


# revision 15
# speedup vs baseline: 1.0214x; 1.0214x over previous
"""Trainium2 Bass kernel: 4-layer MLP (784-512-512-512-10) + log_softmax.

Data-parallel over 8 NeuronCores: batch 65536 is split into 8 shards of
8192 rows; the ~1M-param weights are replicated on every core.

v4 schedule: batch-half-major pipeline with 1-bank PSUM groups.
  * Every (m-chunk, batch-half) matmul group accumulates into its own
    2KB PSUM bank (pool of 8); consumers run one half-block (~3us)
    behind producers, so neither the PE nor its LDWEIGHTS ever waits on
    a ReLU drain (the v3 failure mode: 4 simultaneously-opened 2-bank
    groups stalled the in-order PE queue on PSUM WAR ~1.5us/superchunk
    and the micro-gaps re-throttled the clock to 1.2 GHz).
  * Layer 1's K=16 remainder opens each group via row-tiled
    (tile_position=(32m,0)) matmuls, 4 running concurrently in distinct
    PE row-groups, instead of burning 8 full 512-cycle slots.
  * bias+ReLU alternates ScalarE/DVE per (m, half) so both engines stay
    under the PE's pace; exp/sum-of-exp for superchunk sc-1 is issued
    mid-superchunk (never ahead of ReLUs the PE needs); ln+subtract+
    store happen in two bulk epilogues (ScalarE activation-table swaps
    for LN cost 1.3us each, so they must not recur per superchunk).
  * Layer 4 runs one superchunk behind; all DRAM buffers host-packed
    for contiguous-per-partition DMA; output leaves in the flat SBUF
    layout [128, 64, 10] and is un-permuted on host.

Measured on axon trn2: 172.8us (v1 baseline) -> 155.2us (v3) -> v4.
"""

from contextlib import ExitStack

import ml_dtypes
import numpy as np

import concourse.bass as bass  # noqa: F401  (registers AP machinery)
from concourse import bacc, mybir
from concourse.bass_utils import run_bass_kernel_spmd
from concourse.tile import TileContext

BF16 = mybir.dt.bfloat16
FP32 = mybir.dt.float32
FP8 = mybir.dt.float8e4

N_CORES = 8
B = 65536
D0, H, C = 784, 512, 10
BC = B // N_CORES            # 8192 rows per core
NB = 512                     # matmul moving free dim / PSUM bank width
HB = 2                       # batch halves per superchunk
SNB = NB * HB                # 1024-row superchunk
NCHUNK = BC // SNB           # 8 superchunks
K0F = 6                      # full 128-row contraction chunks in layer 1
K0R = D0 - K0F * 128         # 16 remainder rows
KH = H // 128                # 4 contraction chunks for hidden layers
MG = SNB // 128              # 8 row-groups per superchunk
NRG = BC // 128              # 64 row-groups of 128 rows per core

_CACHED_NC = None


def build_nc():
    nc = bacc.Bacc(
        "TRN2",
        target_bir_lowering=False,
        debug=False,
        enable_asserts=False,
        num_devices=N_CORES,
    )
    xm_d = nc.declare_dram_parameter("xmain", [NCHUNK * 128, K0F * SNB], FP8, isOutput=False)
    xr_d = nc.declare_dram_parameter("xrem", [NCHUNK * 128, SNB], FP8, isOutput=False)
    w1_d = nc.declare_dram_parameter("w1p", [128, K0F * H], FP8, isOutput=False)
    w1r_d = nc.declare_dram_parameter("w1r", [128, 128], FP8, isOutput=False)
    w2_d = nc.declare_dram_parameter("w2p", [128, KH * H], FP8, isOutput=False)
    w3_d = nc.declare_dram_parameter("w3p", [128, KH * H], FP8, isOutput=False)
    w4_d = nc.declare_dram_parameter("w4p", [128, KH * C], BF16, isOutput=False)
    bal_d = nc.declare_dram_parameter("ball", [128, 3 * KH + C], FP32, isOutput=False)
    out_d = nc.declare_dram_parameter("out", [128, NRG, C], FP32, isOutput=True)

    expf = mybir.ActivationFunctionType.Exp
    reluf = mybir.ActivationFunctionType.Relu
    lnf = mybir.ActivationFunctionType.Ln
    add_op = mybir.AluOpType.add
    max_op = mybir.AluOpType.max
    sub_op = mybir.AluOpType.subtract
    drow = mybir.MatmulPerfMode.DoubleRow

    with TileContext(nc) as tc, ExitStack() as ctx:
        consts = ctx.enter_context(tc.tile_pool(name="consts", bufs=1))
        xpool = ctx.enter_context(tc.tile_pool(name="xp", bufs=3))
        hpool = ctx.enter_context(tc.tile_pool(name="hp", bufs=3))
        spool = ctx.enter_context(tc.tile_pool(name="sp", bufs=2))
        pbig = ctx.enter_context(tc.tile_pool(name="pbig", bufs=8, space="PSUM"))

        # Weight-queue order = first-use order: the row-tiled remainder
        # weights open superchunk 0, then w1, then biases (first ReLU is
        # ~2us later), then the rest.
        w1r = consts.tile([128, 128], FP8, tag="w1r", name="w1r")
        nc.scalar.dma_start(w1r[:], w1r_d[:])
        w1 = consts.tile([128, K0F, H], FP8, tag="w1", name="w1")
        for j in range(K0F // 2):
            # k-pair-sized chunks so superchunk 0's kp-major matmuls can
            # start as soon as the first pair lands.
            nc.scalar.dma_start(
                w1[:, 2 * j : 2 * j + 2, :], w1_d[:, j * 2 * H : (j + 1) * 2 * H]
            )
        ball = consts.tile([128, 3 * KH + C], FP32, tag="ball", name="ball")
        nc.scalar.dma_start(ball[:], bal_d[:])
        w2 = consts.tile([128, KH, H], FP8, tag="w2", name="w2")
        nc.scalar.dma_start(w2[:], w2_d[:])
        w3 = consts.tile([128, KH, H], FP8, tag="w3", name="w3")
        nc.scalar.dma_start(w3[:], w3_d[:])
        w4 = consts.tile([128, KH, C], BF16, tag="w4", name="w4")
        nc.scalar.dma_start(w4[:], w4_d[:])

        b4s = ball[:, 3 * KH : 3 * KH + C]

        # PE warm-up: dummy matmuls during the initial DMA wait so the HAM
        # clock gate is at 2.4 GHz when real work arrives.
        warm = consts.tile([128, NB], FP8, tag="warm", name="warm")
        nc.vector.memset(warm[:], 1.0)
        psw = pbig.tile([128, NB], FP32, tag="ps", name="ps_warm")
        for i in range(6):
            nc.tensor.matmul(
                psw[:], lhsT=warm[:, 0:128], rhs=warm[:],
                start=(i == 0), stop=(i == 5),
            )

        # Persistent softmax state: ln+subtract are deferred to two bulk
        # epilogue calls, so ScalarE never swaps activation tables (RELU/EXP
        # share a set, LN does not) inside the steady-state loop.
        logits_all = consts.tile([128, NRG, C], FP32, tag="logits_all", name="logits_all")
        esum_all = consts.tile([128, NRG], FP32, tag="esum_all", name="esum_all")
        lns_all = consts.tile([128, NRG], FP32, tag="lns_all", name="lns_all")
        obuf = consts.tile([128, NRG, C], FP32, tag="obuf", name="obuf")

        def relu_half(ps, out, bias_ap, on_scalar):
            if on_scalar:
                nc.scalar.activation(out, ps[:], reluf, bias=bias_ap)
            else:
                nc.vector.tensor_scalar(out, ps[:], bias_ap, 0.0, add_op, max_op)

        def l4_matmuls(h3, ps4):
            for hb in range(HB):
                for mm in range(NB // 128):
                    r = hb * (NB // 128) + mm
                    ms = slice(mm * 128, (mm + 1) * 128)
                    for k in range(KH):
                        nc.tensor.matmul(
                            ps4[:, r, :], lhsT=h3[k][:, hb, ms], rhs=w4[:, k, :],
                            start=(k == 0), stop=(k == KH - 1),
                        )

        def l4_softmax_state(sc, ps4):
            # logits + exp + sum(exp) for superchunk sc.
            rg0 = sc * MG
            lg = logits_all[:, rg0 : rg0 + MG, :]
            nc.vector.tensor_tensor(
                lg, ps4[:, 0:MG, :],
                b4s[:, None, :].to_broadcast((128, MG, C)), add_op,
            )
            etile = spool.tile([128, MG, C], FP32, tag="etile", name="etile")
            nc.scalar.activation(etile[:], lg, expf)
            nc.vector.tensor_reduce(
                esum_all[:, rg0 : rg0 + MG], etile[:],
                axis=mybir.AxisListType.X, op=add_op,
            )

        def softmax_epilogue(rg0, rg1):
            # out = logits - ln(sum(exp(logits))) for row-groups [rg0, rg1)
            n = rg1 - rg0
            nc.scalar.activation(lns_all[:, rg0:rg1], esum_all[:, rg0:rg1], lnf)
            nc.vector.tensor_tensor(
                obuf[:, rg0:rg1, :], logits_all[:, rg0:rg1, :],
                lns_all[:, rg0:rg1, None].to_broadcast((128, n, C)), sub_op,
            )
            nc.sync.dma_start(out_d[:, rg0:rg1, :], obuf[:, rg0:rg1, :])

        h3_prev = None
        ps4_prev = None

        def dma_x(sc):
            xr = xpool.tile([128, SNB], FP8, tag="xr", name="xr")
            nc.sync.dma_start(xr[:], xr_d[sc * 128 : (sc + 1) * 128, :])
            xt = xpool.tile([128, K0F, SNB], FP8, tag="xt", name="xt")
            for j in range(K0F // 2):
                nc.sync.dma_start(
                    xt[:, 2 * j : 2 * j + 2, :],
                    xm_d[sc * 128 : (sc + 1) * 128, j * 2 * SNB : (j + 1) * 2 * SNB],
                )
            return xr, xt

        def l1_rem(ps1, xr, hb):
            # K=16 remainder opens all 4 m-groups concurrently (distinct PE
            # row-groups).
            bsl = slice(hb * NB, (hb + 1) * NB)
            for m in range(KH):
                nc.tensor.matmul(
                    ps1[m][:], lhsT=w1r[32 * m : 32 * m + K0R, :],
                    rhs=xr[32 * m : 32 * m + K0R, bsl],
                    start=True, stop=False, perf_mode=None,
                    tile_position=(32 * m, 0),
                )

        def alloc_ps1(hb):
            return [
                pbig.tile([128, NB], FP32, tag="ps", name=f"ps1_{m}_{hb}")
                for m in range(KH)
            ]

        nonlocal_state = {"x_next": dma_x(0), "ps1h0_next": None}

        for sc in range(NCHUNK):
            xr, xt = nonlocal_state["x_next"]
            ps1h0_next = nonlocal_state["ps1h0_next"]

            # Layer 1 [784 -> 512], one batch-half at a time. The h0
            # remainder matmuls were issued early (mid-previous-superchunk)
            # so their PSUM WAR deps are long clear and they stay 4-way
            # concurrent; superchunk 0 runs kp-major so it can start on the
            # first-arriving x/w1 k-pair chunk.
            h1p = [
                hpool.tile([128, 2, HB, NB], FP8, tag=f"h1p_{j}", name=f"h1p_{j}")
                for j in range(KH // 2)
            ]
            for hb in range(HB):
                bsl = slice(hb * NB, (hb + 1) * NB)
                if hb == 0 and ps1h0_next is not None:
                    ps1 = ps1h0_next
                else:
                    ps1 = alloc_ps1(hb)
                    l1_rem(ps1, xr, hb)
                if sc == 0:
                    for k in range(0, K0F, 2):
                        for m in range(KH):
                            ms = slice(m * 128, (m + 1) * 128)
                            nc.tensor.matmul(
                                ps1[m][:], lhsT=w1[:, k : k + 2, ms],
                                rhs=xt[:, k : k + 2, bsl],
                                start=False, stop=(k == K0F - 2), perf_mode=drow,
                            )
                else:
                    for m in range(KH):
                        ms = slice(m * 128, (m + 1) * 128)
                        for k in range(0, K0F, 2):
                            nc.tensor.matmul(
                                ps1[m][:], lhsT=w1[:, k : k + 2, ms],
                                rhs=xt[:, k : k + 2, bsl],
                                start=False, stop=(k == K0F - 2), perf_mode=drow,
                            )
                for m in range(KH):
                    relu_half(
                        ps1[m], h1p[m // 2][:, m % 2, hb, :],
                        ball[:, m : m + 1], on_scalar=((m + hb) % 2 == 0),
                    )

            # Layer 4 of the previous superchunk (its inputs are long ready).
            if h3_prev is not None:
                l4_matmuls(h3_prev, ps4_prev)

            def hidden_layer(w, src, dsts, bias_base, out_of_h3):
                for hb in range(HB):
                    ps = [
                        pbig.tile([128, NB], FP32, tag="ps", name=f"psh_{m}_{hb}")
                        for m in range(KH)
                    ]
                    for m in range(KH):
                        ms = slice(m * 128, (m + 1) * 128)
                        for j in range(KH // 2):
                            nc.tensor.matmul(
                                ps[m][:], lhsT=w[:, 2 * j : 2 * j + 2, ms],
                                rhs=src[j][:, :, hb, :],
                                start=(j == 0), stop=(j == KH // 2 - 1),
                                perf_mode=drow,
                            )
                        out = (
                            dsts[m][:, hb, :] if out_of_h3
                            else dsts[m // 2][:, m % 2, hb, :]
                        )
                        relu_half(
                            ps[m], out, ball[:, bias_base + m : bias_base + m + 1],
                            on_scalar=((m + hb) % 2 == 0),
                        )
                    if hb == 0 and out_of_h3 is False and h3_prev is not None:
                        # exp/sum(exp) of the previous superchunk: issued
                        # mid-superchunk so its ScalarE/DVE ops never queue
                        # ahead of ReLUs the PE is about to wait on.
                        l4_softmax_state(sc - 1, ps4_prev)
                        if sc == NCHUNK - 1:
                            # Bulk ln+subtract+store for superchunks 0-6;
                            # the L3 consumers of the ReLUs this delays are
                            # a full half-block behind, so the activation-
                            # table swap hides here.
                            softmax_epilogue(0, (NCHUNK - 1) * MG)
                    if hb == 0 and out_of_h3 and sc < NCHUNK - 1:
                        # Prefetch next superchunk's x and open its L1-h0
                        # groups now: the PSUM buffers these claim drained
                        # ~4us ago, so the remainder matmuls issue wait-free
                        # and 4-way concurrent.
                        nonlocal_state["x_next"] = dma_x(sc + 1)
                        ps1n = alloc_ps1(0)
                        l1_rem(ps1n, nonlocal_state["x_next"][0], 0)
                        nonlocal_state["ps1h0_next"] = ps1n

            # Layer 2 [512 -> 512]
            h2p = [
                hpool.tile([128, 2, HB, NB], FP8, tag=f"h2p_{j}", name=f"h2p_{j}")
                for j in range(KH // 2)
            ]
            hidden_layer(w2, h1p, h2p, KH, out_of_h3=False)

            # Layer 3 [512 -> 512], bf16 out (layer-4 lhsT)
            h3 = [
                hpool.tile([128, HB, NB], BF16, tag=f"h3_{m}", name=f"h3_{m}")
                for m in range(KH)
            ]
            hidden_layer(w3, h2p, h3, 2 * KH, out_of_h3=True)

            h3_prev = h3
            ps4_prev = pbig.tile([128, MG, C], FP32, tag="ps", name="ps4")

        l4_matmuls(h3_prev, ps4_prev)
        l4_softmax_state(NCHUNK - 1, ps4_prev)
        softmax_epilogue((NCHUNK - 1) * MG, NRG)

    nc.compile()
    return nc


def _get_nc():
    global _CACHED_NC
    if _CACHED_NC is None:
        _CACHED_NC = build_nc()
    return _CACHED_NC


def make_in_maps(x, W1, b1, W2, b2, W3, b3, W4, b4):
    bf16 = ml_dtypes.bfloat16
    fp8 = ml_dtypes.float8_e4m3
    f32 = np.float32
    W1, W2, W3, W4 = (np.asarray(w, dtype=f32) for w in (W1, W2, W3, W4))

    # w1p[p, k*512+m] = W1[m, k*128+p]
    w1p = np.ascontiguousarray(
        W1[:, : K0F * 128].reshape(H, K0F, 128).transpose(2, 1, 0)
    ).reshape(128, K0F * H).astype(fp8)
    # w1r[32i+j, c] = W1[128i+c, 768+j]  (row-tiled remainder weights)
    w1r = np.zeros((128, 128), dtype=fp8)
    wr = W1[:, K0F * 128 :].astype(fp8)  # [512, 16]
    for i in range(KH):
        w1r[32 * i : 32 * i + K0R, :] = wr[128 * i : 128 * (i + 1), :].T
    # w2p[p, o*512+m] = W2[m, o*128+p]
    def packw(W):
        return np.ascontiguousarray(
            W.T.reshape(KH, 128, H).transpose(1, 0, 2)
        ).reshape(128, KH * H).astype(fp8)
    w2p, w3p = packw(W2), packw(W3)
    w4p = np.ascontiguousarray(
        W4.T.reshape(KH, 128, C).transpose(1, 0, 2)
    ).reshape(128, KH * C).astype(bf16)
    ball = np.concatenate(
        [
            np.asarray(b1, f32).reshape(KH, 128).T,
            np.asarray(b2, f32).reshape(KH, 128).T,
            np.asarray(b3, f32).reshape(KH, 128).T,
            np.tile(np.asarray(b4, f32)[None, :], (128, 1)),
        ],
        axis=1,
    )
    common = {
        "w1p": w1p, "w1r": w1r, "w2p": w2p, "w3p": w3p, "w4p": w4p,
        "ball": np.ascontiguousarray(ball),
    }

    xq = np.asarray(x).astype(fp8)
    in_maps = []
    for ci in range(N_CORES):
        xs = xq[ci * BC : (ci + 1) * BC]  # [8192, 784]
        # xmain[sc*128+p, k*1024+b] = xs[sc*1024+b, k*128+p]
        xmain = np.ascontiguousarray(
            xs[:, : K0F * 128].reshape(NCHUNK, SNB, K0F, 128).transpose(0, 3, 2, 1)
        ).reshape(NCHUNK * 128, K0F * SNB)
        # xrem[sc*128+32i+j, b] = xs[sc*1024+b, 768+j], replicated over i
        xrp = xs[:, K0F * 128 :].reshape(NCHUNK, SNB, K0R).transpose(0, 2, 1)
        xrem = np.zeros((NCHUNK, 128, SNB), dtype=fp8)
        for i in range(KH):
            xrem[:, 32 * i : 32 * i + K0R, :] = xrp
        in_maps.append(
            {"xmain": xmain, "xrem": xrem.reshape(NCHUNK * 128, SNB), **common}
        )
    return in_maps


def assemble_output(res):
    # out dram is the flat SBUF layout [128, 64, 10]; row rg*128+p of the
    # core's shard lives at out[p, rg, :].
    parts = []
    for i in range(N_CORES):
        o = np.asarray(res.results[i]["out"], dtype=np.float32)
        parts.append(o.transpose(1, 0, 2).reshape(BC, C))
    return np.concatenate(parts, axis=0)


def kernel(x, W1, b1, W2, b2, W3, b3, W4, b4):
    in_maps = make_in_maps(x, W1, b1, W2, b2, W3, b3, W4, b4)
    nc = _get_nc()
    res = run_bass_kernel_spmd(nc, in_maps, list(range(N_CORES)))
    return assemble_output(res)


# revision 20
# speedup vs baseline: 1.0249x; 1.0035x over previous
"""Trainium2 Bass kernel: 4-layer MLP (784-512-512-512-10) + log_softmax.

Data-parallel over 8 NeuronCores: batch 65536 is split into 8 shards of
8192 rows; the ~1M-param weights are replicated on every core.

v4 schedule: batch-half-major pipeline with 1-bank PSUM groups.
  * Every (m-chunk, batch-half) matmul group accumulates into its own
    2KB PSUM bank (pool of 8); consumers run one half-block (~3us)
    behind producers, so neither the PE nor its LDWEIGHTS ever waits on
    a ReLU drain (the v3 failure mode: 4 simultaneously-opened 2-bank
    groups stalled the in-order PE queue on PSUM WAR ~1.5us/superchunk
    and the micro-gaps re-throttled the clock to 1.2 GHz).
  * Layer 1's K=16 remainder opens each group via row-tiled
    (tile_position=(32m,0)) matmuls, 4 running concurrently in distinct
    PE row-groups, instead of burning 8 full 512-cycle slots.
  * bias+ReLU alternates ScalarE/DVE per (m, half) so both engines stay
    under the PE's pace; exp/sum-of-exp for superchunk sc-1 is issued
    mid-superchunk (never ahead of ReLUs the PE needs); ln+subtract+
    store happen in two bulk epilogues (ScalarE activation-table swaps
    for LN cost 1.3us each, so they must not recur per superchunk).
  * Layer 4 runs one superchunk behind; all DRAM buffers host-packed
    for contiguous-per-partition DMA; output leaves in the flat SBUF
    layout [128, 64, 10] and is un-permuted on host.

Measured on axon trn2: 172.8us (v1 baseline) -> 155.2us (v3) -> v4.
"""

from contextlib import ExitStack

import ml_dtypes
import numpy as np

import concourse.bass as bass  # noqa: F401  (registers AP machinery)
from concourse import bacc, mybir
from concourse.bass_utils import run_bass_kernel_spmd
from concourse.tile import TileContext

BF16 = mybir.dt.bfloat16
FP32 = mybir.dt.float32
FP8 = mybir.dt.float8e4

N_CORES = 8
B = 65536
D0, H, C = 784, 512, 10
BC = B // N_CORES            # 8192 rows per core
NB = 512                     # matmul moving free dim / PSUM bank width
HB = 2                       # batch halves per superchunk
SNB = NB * HB                # 1024-row superchunk
NCHUNK = BC // SNB           # 8 superchunks
K0F = 6                      # full 128-row contraction chunks in layer 1
K0R = D0 - K0F * 128         # 16 remainder rows
KH = H // 128                # 4 contraction chunks for hidden layers
MG = SNB // 128              # 8 row-groups per superchunk
NRG = BC // 128              # 64 row-groups of 128 rows per core

_CACHED_NC = None


def build_nc():
    nc = bacc.Bacc(
        "TRN2",
        target_bir_lowering=False,
        debug=False,
        enable_asserts=False,
        num_devices=N_CORES,
    )
    xm_d = nc.declare_dram_parameter("xmain", [NCHUNK * 128, K0F * SNB], FP8, isOutput=False)
    xr_d = nc.declare_dram_parameter("xrem", [NCHUNK * 128, SNB], FP8, isOutput=False)
    w1_d = nc.declare_dram_parameter("w1p", [128, K0F * H], FP8, isOutput=False)
    w1r_d = nc.declare_dram_parameter("w1r", [128, 128], FP8, isOutput=False)
    w2_d = nc.declare_dram_parameter("w2p", [128, KH * H], FP8, isOutput=False)
    w3_d = nc.declare_dram_parameter("w3p", [128, KH * H], FP8, isOutput=False)
    w4_d = nc.declare_dram_parameter("w4p", [128, KH * C], BF16, isOutput=False)
    bal_d = nc.declare_dram_parameter("ball", [128, 3 * KH + C], FP32, isOutput=False)
    out_d = nc.declare_dram_parameter("out", [128, NRG, C], FP32, isOutput=True)

    expf = mybir.ActivationFunctionType.Exp
    reluf = mybir.ActivationFunctionType.Relu
    lnf = mybir.ActivationFunctionType.Ln
    add_op = mybir.AluOpType.add
    max_op = mybir.AluOpType.max
    sub_op = mybir.AluOpType.subtract
    mult_op = mybir.AluOpType.mult
    drow = mybir.MatmulPerfMode.DoubleRow

    with TileContext(nc) as tc, ExitStack() as ctx:
        consts = ctx.enter_context(tc.tile_pool(name="consts", bufs=1))
        xpool = ctx.enter_context(tc.tile_pool(name="xp", bufs=3))
        hpool = ctx.enter_context(tc.tile_pool(name="hp", bufs=3))
        spool = ctx.enter_context(tc.tile_pool(name="sp", bufs=2))
        pbig = ctx.enter_context(tc.tile_pool(name="pbig", bufs=8, space="PSUM"))

        # Weight-queue order = first-use order: the row-tiled remainder
        # weights open superchunk 0, then w1, then biases (first ReLU is
        # ~2us later), then the rest.
        w1r = consts.tile([128, 128], FP8, tag="w1r", name="w1r")
        nc.scalar.dma_start(w1r[:], w1r_d[:])
        w1 = consts.tile([128, K0F, H], FP8, tag="w1", name="w1")
        for j in range(K0F // 2):
            # k-pair-sized chunks so superchunk 0's kp-major matmuls can
            # start as soon as the first pair lands.
            nc.scalar.dma_start(
                w1[:, 2 * j : 2 * j + 2, :], w1_d[:, j * 2 * H : (j + 1) * 2 * H]
            )
        ball = consts.tile([128, 3 * KH + C], FP32, tag="ball", name="ball")
        nc.scalar.dma_start(ball[:], bal_d[:])
        w2 = consts.tile([128, KH, H], FP8, tag="w2", name="w2")
        nc.scalar.dma_start(w2[:], w2_d[:])
        w3 = consts.tile([128, KH, H], FP8, tag="w3", name="w3")
        nc.scalar.dma_start(w3[:], w3_d[:])
        w4 = consts.tile([128, KH, C], BF16, tag="w4", name="w4")
        nc.scalar.dma_start(w4[:], w4_d[:])

        b4s = ball[:, 3 * KH : 3 * KH + C]

        # PE warm-up: dummy matmuls during the initial DMA wait so the HAM
        # clock gate is at 2.4 GHz when real work arrives.
        warm = consts.tile([128, NB], FP8, tag="warm", name="warm")
        nc.vector.memset(warm[:], 1.0)
        negone = consts.tile([128, 1], FP32, tag="negone", name="negone")
        nc.vector.memset(negone[:], -1.0)
        psw = pbig.tile([128, NB], FP32, tag="ps", name="ps_warm")
        for i in range(6):
            nc.tensor.matmul(
                psw[:], lhsT=warm[:, 0:128], rhs=warm[:],
                start=(i == 0), stop=(i == 5),
            )

        # Persistent softmax state: ln+subtract are deferred to two bulk
        # epilogue calls, so ScalarE never swaps activation tables (RELU/EXP
        # share a set, LN does not) inside the steady-state loop.
        logits_all = consts.tile([128, NRG, C], FP32, tag="logits_all", name="logits_all")
        esum_all = consts.tile([128, NRG], FP32, tag="esum_all", name="esum_all")
        lns_all = consts.tile([128, NRG], FP32, tag="lns_all", name="lns_all")
        obuf = consts.tile([128, NRG, C], FP32, tag="obuf", name="obuf")

        def relu_half(ps, out, bias_ap, on_scalar):
            if on_scalar:
                nc.scalar.activation(out, ps[:], reluf, bias=bias_ap)
            else:
                nc.vector.tensor_scalar(out, ps[:], bias_ap, 0.0, add_op, max_op)

        def l4_matmuls(h3, ps4):
            for hb in range(HB):
                for mm in range(NB // 128):
                    r = hb * (NB // 128) + mm
                    ms = slice(mm * 128, (mm + 1) * 128)
                    for k in range(KH):
                        nc.tensor.matmul(
                            ps4[:, r, :], lhsT=h3[k][:, hb, ms], rhs=w4[:, k, :],
                            start=(k == 0), stop=(k == KH - 1),
                        )

        def l4_softmax_state(sc, ps4):
            # logits + exp + sum(exp) for superchunk sc.
            rg0 = sc * MG
            lg = logits_all[:, rg0 : rg0 + MG, :]
            nc.vector.tensor_tensor(
                lg, ps4[:, 0:MG, :],
                b4s[:, None, :].to_broadcast((128, MG, C)), add_op,
            )
            etile = spool.tile([128, MG, C], FP32, tag="etile", name="etile")
            nc.scalar.activation(etile[:], lg, expf)
            nc.vector.tensor_reduce(
                esum_all[:, rg0 : rg0 + MG], etile[:],
                axis=mybir.AxisListType.X, op=add_op,
            )

        def softmax_epilogue(rg0, rg1):
            # out = logits - ln(sum(exp(logits))) for row-groups [rg0, rg1).
            # ln is computed WITHOUT the Ln activation (whose table set
            # excludes Exp, so each use would cost two 1.28us table swaps):
            # y0 = bits(s)*ln2/2^23 - 126.9427*ln2 - 1 (Mitchell estimate,
            # pre-decremented), then one Newton step ln(s) ~ y0 + s*e^(-y0-1)
            # via the Exp activation that shares the ReLU/Exp table.
            n = rg1 - rg0
            # Private copy first: every fast-ln op then reads DVE-locally
            # written tiles (bitcast views of cross-engine subtile writes
            # are not trusted by the dep tracker).
            esc = spool.tile([128, n], FP32, tag=f"esc_{rg0}", name="esc")
            nc.vector.tensor_copy(esc[:], esum_all[:, rg0:rg1])
            y0 = spool.tile([128, n], FP32, tag=f"y0_{rg0}", name="y0")
            nc.vector.tensor_scalar(
                y0[:], esc[:].bitcast(mybir.dt.int32),
                8.2629582e-8, -88.98996728, mult_op, add_op,
            )
            u = spool.tile([128, n], FP32, tag=f"u_{rg0}", name="u")
            nc.scalar.activation(u[:], y0[:], expf, bias=negone[:, 0:1], scale=-1.0)
            v = spool.tile([128, n], FP32, tag=f"v_{rg0}", name="v")
            nc.vector.tensor_tensor(v[:], esc[:], u[:], mult_op)
            nc.vector.tensor_tensor(lns_all[:, rg0:rg1], y0[:], v[:], add_op)
            nc.vector.tensor_tensor(
                obuf[:, rg0:rg1, :], logits_all[:, rg0:rg1, :],
                lns_all[:, rg0:rg1, None].to_broadcast((128, n, C)), sub_op,
            )
            nc.sync.dma_start(out_d[:, rg0:rg1, :], obuf[:, rg0:rg1, :])

        h3_prev = None
        ps4_prev = None

        def dma_x(sc):
            xr = xpool.tile([128, SNB], FP8, tag="xr", name="xr")
            nc.sync.dma_start(xr[:], xr_d[sc * 128 : (sc + 1) * 128, :])
            xt = xpool.tile([128, K0F, SNB], FP8, tag="xt", name="xt")
            for j in range(K0F // 2):
                nc.sync.dma_start(
                    xt[:, 2 * j : 2 * j + 2, :],
                    xm_d[sc * 128 : (sc + 1) * 128, j * 2 * SNB : (j + 1) * 2 * SNB],
                )
            return xr, xt

        def l1_rem(ps1, xr, hb):
            # K=16 remainder opens all 4 m-groups concurrently (distinct PE
            # row-groups).
            bsl = slice(hb * NB, (hb + 1) * NB)
            for m in range(KH):
                nc.tensor.matmul(
                    ps1[m][:], lhsT=w1r[32 * m : 32 * m + K0R, :],
                    rhs=xr[32 * m : 32 * m + K0R, bsl],
                    start=True, stop=False, perf_mode=None,
                    tile_position=(32 * m, 0),
                )

        def alloc_ps1(hb):
            return [
                pbig.tile([128, NB], FP32, tag="ps", name=f"ps1_{m}_{hb}")
                for m in range(KH)
            ]

        nonlocal_state = {"x_next": dma_x(0), "ps1h0_next": None}

        for sc in range(NCHUNK):
            xr, xt = nonlocal_state["x_next"]
            ps1h0_next = nonlocal_state["ps1h0_next"]

            # Layer 1 [784 -> 512], one batch-half at a time. The h0
            # remainder matmuls were issued early (mid-previous-superchunk)
            # so their PSUM WAR deps are long clear and they stay 4-way
            # concurrent; superchunk 0 runs kp-major so it can start on the
            # first-arriving x/w1 k-pair chunk.
            h1p = [
                hpool.tile([128, 2, HB, NB], FP8, tag=f"h1p_{j}", name=f"h1p_{j}")
                for j in range(KH // 2)
            ]
            for hb in range(HB):
                bsl = slice(hb * NB, (hb + 1) * NB)
                if hb == 0 and ps1h0_next is not None:
                    ps1 = ps1h0_next
                else:
                    ps1 = alloc_ps1(hb)
                    l1_rem(ps1, xr, hb)
                if sc == 0:
                    for k in range(0, K0F, 2):
                        for m in range(KH):
                            ms = slice(m * 128, (m + 1) * 128)
                            nc.tensor.matmul(
                                ps1[m][:], lhsT=w1[:, k : k + 2, ms],
                                rhs=xt[:, k : k + 2, bsl],
                                start=False, stop=(k == K0F - 2), perf_mode=drow,
                            )
                else:
                    for m in range(KH):
                        ms = slice(m * 128, (m + 1) * 128)
                        for k in range(0, K0F, 2):
                            nc.tensor.matmul(
                                ps1[m][:], lhsT=w1[:, k : k + 2, ms],
                                rhs=xt[:, k : k + 2, bsl],
                                start=False, stop=(k == K0F - 2), perf_mode=drow,
                            )
                for m in range(KH):
                    relu_half(
                        ps1[m], h1p[m // 2][:, m % 2, hb, :],
                        ball[:, m : m + 1], on_scalar=((m + hb) % 2 == 0),
                    )

            # Layer 4 of the previous superchunk (its inputs are long ready).
            if h3_prev is not None:
                l4_matmuls(h3_prev, ps4_prev)

            def hidden_layer(w, src, dsts, bias_base, out_of_h3):
                for hb in range(HB):
                    ps = [
                        pbig.tile([128, NB], FP32, tag="ps", name=f"psh_{m}_{hb}")
                        for m in range(KH)
                    ]
                    for m in range(KH):
                        ms = slice(m * 128, (m + 1) * 128)
                        for j in range(KH // 2):
                            nc.tensor.matmul(
                                ps[m][:], lhsT=w[:, 2 * j : 2 * j + 2, ms],
                                rhs=src[j][:, :, hb, :],
                                start=(j == 0), stop=(j == KH // 2 - 1),
                                perf_mode=drow,
                            )
                        out = (
                            dsts[m][:, hb, :] if out_of_h3
                            else dsts[m // 2][:, m % 2, hb, :]
                        )
                        relu_half(
                            ps[m], out, ball[:, bias_base + m : bias_base + m + 1],
                            on_scalar=((m + hb) % 2 == 0),
                        )
                    if hb == 0 and out_of_h3 is False and h3_prev is not None:
                        # exp/sum(exp) of the previous superchunk: issued
                        # mid-superchunk so its ScalarE/DVE ops never queue
                        # ahead of ReLUs the PE is about to wait on.
                        l4_softmax_state(sc - 1, ps4_prev)
                        if sc == NCHUNK - 1:
                            # Bulk ln+subtract+store for superchunks 0-6;
                            # the L3 consumers of the ReLUs this delays are
                            # a full half-block behind, so the activation-
                            # table swap hides here.
                            softmax_epilogue(0, (NCHUNK - 1) * MG)
                    if hb == 0 and out_of_h3 and sc < NCHUNK - 1:
                        # Prefetch next superchunk's x and open its L1-h0
                        # groups now: the PSUM buffers these claim drained
                        # ~4us ago, so the remainder matmuls issue wait-free
                        # and 4-way concurrent.
                        nonlocal_state["x_next"] = dma_x(sc + 1)
                        ps1n = alloc_ps1(0)
                        l1_rem(ps1n, nonlocal_state["x_next"][0], 0)
                        nonlocal_state["ps1h0_next"] = ps1n

            # Layer 2 [512 -> 512]
            h2p = [
                hpool.tile([128, 2, HB, NB], FP8, tag=f"h2p_{j}", name=f"h2p_{j}")
                for j in range(KH // 2)
            ]
            hidden_layer(w2, h1p, h2p, KH, out_of_h3=False)

            # Layer 3 [512 -> 512], bf16 out (layer-4 lhsT)
            h3 = [
                hpool.tile([128, HB, NB], BF16, tag=f"h3_{m}", name=f"h3_{m}")
                for m in range(KH)
            ]
            hidden_layer(w3, h2p, h3, 2 * KH, out_of_h3=True)

            h3_prev = h3
            ps4_prev = pbig.tile([128, MG, C], FP32, tag="ps", name="ps4")

        l4_matmuls(h3_prev, ps4_prev)
        l4_softmax_state(NCHUNK - 1, ps4_prev)
        softmax_epilogue((NCHUNK - 1) * MG, NRG)

    nc.compile()
    return nc


def _get_nc():
    global _CACHED_NC
    if _CACHED_NC is None:
        _CACHED_NC = build_nc()
    return _CACHED_NC


def make_in_maps(x, W1, b1, W2, b2, W3, b3, W4, b4):
    bf16 = ml_dtypes.bfloat16
    fp8 = ml_dtypes.float8_e4m3
    f32 = np.float32
    W1, W2, W3, W4 = (np.asarray(w, dtype=f32) for w in (W1, W2, W3, W4))

    # w1p[p, k*512+m] = W1[m, k*128+p]
    w1p = np.ascontiguousarray(
        W1[:, : K0F * 128].reshape(H, K0F, 128).transpose(2, 1, 0)
    ).reshape(128, K0F * H).astype(fp8)
    # w1r[32i+j, c] = W1[128i+c, 768+j]  (row-tiled remainder weights)
    w1r = np.zeros((128, 128), dtype=fp8)
    wr = W1[:, K0F * 128 :].astype(fp8)  # [512, 16]
    for i in range(KH):
        w1r[32 * i : 32 * i + K0R, :] = wr[128 * i : 128 * (i + 1), :].T
    # w2p[p, o*512+m] = W2[m, o*128+p]
    def packw(W):
        return np.ascontiguousarray(
            W.T.reshape(KH, 128, H).transpose(1, 0, 2)
        ).reshape(128, KH * H).astype(fp8)
    w2p, w3p = packw(W2), packw(W3)
    w4p = np.ascontiguousarray(
        W4.T.reshape(KH, 128, C).transpose(1, 0, 2)
    ).reshape(128, KH * C).astype(bf16)
    ball = np.concatenate(
        [
            np.asarray(b1, f32).reshape(KH, 128).T,
            np.asarray(b2, f32).reshape(KH, 128).T,
            np.asarray(b3, f32).reshape(KH, 128).T,
            np.tile(np.asarray(b4, f32)[None, :], (128, 1)),
        ],
        axis=1,
    )
    common = {
        "w1p": w1p, "w1r": w1r, "w2p": w2p, "w3p": w3p, "w4p": w4p,
        "ball": np.ascontiguousarray(ball),
    }

    xq = np.asarray(x).astype(fp8)
    in_maps = []
    for ci in range(N_CORES):
        xs = xq[ci * BC : (ci + 1) * BC]  # [8192, 784]
        # xmain[sc*128+p, k*1024+b] = xs[sc*1024+b, k*128+p]
        xmain = np.ascontiguousarray(
            xs[:, : K0F * 128].reshape(NCHUNK, SNB, K0F, 128).transpose(0, 3, 2, 1)
        ).reshape(NCHUNK * 128, K0F * SNB)
        # xrem[sc*128+32i+j, b] = xs[sc*1024+b, 768+j], replicated over i
        xrp = xs[:, K0F * 128 :].reshape(NCHUNK, SNB, K0R).transpose(0, 2, 1)
        xrem = np.zeros((NCHUNK, 128, SNB), dtype=fp8)
        for i in range(KH):
            xrem[:, 32 * i : 32 * i + K0R, :] = xrp
        in_maps.append(
            {"xmain": xmain, "xrem": xrem.reshape(NCHUNK * 128, SNB), **common}
        )
    return in_maps


def assemble_output(res):
    # out dram is the flat SBUF layout [128, 64, 10]; row rg*128+p of the
    # core's shard lives at out[p, rg, :].
    parts = []
    for i in range(N_CORES):
        o = np.asarray(res.results[i]["out"], dtype=np.float32)
        parts.append(o.transpose(1, 0, 2).reshape(BC, C))
    return np.concatenate(parts, axis=0)


def kernel(x, W1, b1, W2, b2, W3, b3, W4, b4):
    in_maps = make_in_maps(x, W1, b1, W2, b2, W3, b3, W4, b4)
    nc = _get_nc()
    res = run_bass_kernel_spmd(nc, in_maps, list(range(N_CORES)))
    return assemble_output(res)
